# revision 3
# baseline (speedup 1.0000x reference)
"""Segment softmax (GAT attention stage 4) on 8 TRN2 NeuronCores — v10: all-bf16 interleave, periodic-AP multiply everywhere.

alpha_i = exp(e_i) / sum_{j: tgt_j == tgt_i} exp(e_j)

Mathematically identical to the reference (max-subtraction cancels; with
e ~ N(0,1) nothing can overflow f32 and the +1e-16 regularizer is
negligible against segment sums of ~256 terms).

Node-sharded across the 8 cores (no collective), two-tier padding
(7168 highest-degree nodes at W2=352, rest at W1=280), fp16 in /
bf16 out. v7 switches each group to an INTERLEAVED column layout:
column c = wcol*nb + b (node-block innermost). Consequences:

  - the per-node reduce becomes log2 fold-halving adds over FLAT
    CONTIGUOUS slices — the only AP shape the DVE 16-bit 2x mode
    engages for — plus a tiny transposed-AP 1x tail reduce
  - exp stays one big contiguous ACT instr (layout-agnostic)
  - the scale-multiply uses a periodic in1 AP ([P, w, nb], middle
    stride 0, last dim packed) on some groups and a materialized
    repeated-r tile with guaranteed-2x contiguous chunks on others
    (A/B probe; both are correct), with a few chunks on the idle
    Pool engine to shave DVE load.

Falls back to a flat layout if the degree distribution violates the
tier bounds (harness data never does).
"""

import numpy as np

P = 128
NCORES = 8
NUM_NODES = 100_000
BLOCKS_PER_CORE = 98
NPAD = NCORES * BLOCKS_PER_CORE * P

W1, T1, NB1, NG1 = 280, 91, 7, 13  # light tier: 13 groups of 7 blocks
W2, T2, NB2, NG2 = 352, 7, 7, 1  # heavy tier: 1 group of 7 blocks
K2 = NCORES * T2 * P
K1 = NCORES * T1 * P
KREP = {280: 70, 352: 88}  # rep-tile width (chunk = KREP*nb cols)

_CACHE = {}


def _emit_tier(nc, pool, mybir, x_in, a_out, ng, nb, w, periodic_groups,
               pool_chunks, split_first):
    for g in range(ng):
        xt = pool.tile([P, nb * w], mybir.dt.float16, tag="x")
        if split_first and g == 0:
            h = nb * (w // 2)
            nc.sync.dma_start(out=xt[:, :h], in_=x_in[g][:, :h])
            nc.sync.dma_start(out=xt[:, h:], in_=x_in[g][:, h:])
        else:
            nc.sync.dma_start(out=xt[:], in_=x_in[g])
        yt = pool.tile([P, nb * w], mybir.dt.bfloat16, tag="y")
        nc.scalar.activation(yt[:], xt[:], mybir.ActivationFunctionType.Exp)

        # fold-halving reduce on flat contiguous slices (DVE 16-bit 2x);
        # fp16 partials: values <= ~4e3, eps 2^-11 -> ~5e-4 relative on sums
        half = w // 2
        zt = pool.tile([P, nb * half], mybir.dt.bfloat16, tag="z")
        with nc.allow_low_precision(reason="bf16 fold partials, gate 2e-2"):
            nc.vector.tensor_add(
                out=zt[:, : nb * half], in0=yt[:, : nb * half],
                in1=yt[:, nb * half :],
            )
            width = half
            while width > 22 and width % 2 == 0:
                width //= 2
                nc.vector.tensor_add(
                    out=zt[:, : nb * width],
                    in0=zt[:, : nb * width],
                    in1=zt[:, nb * width : nb * 2 * width],
                )
        # tail: [P, width, nb] -> transpose AP -> per-node sums (f32, 1x)
        st = pool.tile([P, nb], mybir.dt.float32, tag="s")
        ztail = zt[:, : nb * width].rearrange("p (w b) -> p w b", b=nb)
        nc.vector.tensor_reduce(
            out=st[:], in_=ztail.transpose([0, 2, 1]),
            axis=mybir.AxisListType.X, op=mybir.AluOpType.add,
        )
        rt = pool.tile([P, nb], mybir.dt.bfloat16, tag="r")
        with nc.allow_low_precision(reason="bf16 r, gate 2e-2"):
            nc.vector.reciprocal(out=rt[:], in_=st[:])

        if g in periodic_groups:
            # one multiply, in1 = r through a periodic AP (2x probe)
            yv = yt[:].rearrange("p (w b) -> p w b", b=nb)
            rb = rt[:].unsqueeze(1).broadcast_to([P, w, nb])
            nc.vector.tensor_mul(out=yv, in0=yv, in1=rb)
        else:
            # materialize r repeated KREP times; guaranteed-2x chunks
            k = KREP[w]
            rep = pool.tile([P, k * nb], mybir.dt.float16, tag="rr")
            rbk = rt[:].unsqueeze(1).broadcast_to([P, k, nb])
            nc.vector.tensor_scalar_mul(
                out=rep[:].rearrange("p (w b) -> p w b", b=nb),
                in0=rbk, scalar1=1.0,
            )
            nchunk = w // k
            for c in range(nchunk):
                blk = yt[:, c * k * nb : (c + 1) * k * nb]
                nc.vector.tensor_mul(out=blk, in0=blk, in1=rep[:])
        nc.sync.dma_start(out=a_out[g], in_=yt[:])


def _build_two_tier():
    import concourse.mybir as mybir
    from concourse import bacc
    from concourse.tile import TileContext

    nc = bacc.Bacc(None, target_bir_lowering=False)
    xl = nc.dram_tensor(
        "xl", [NG1, P, NB1 * W1], mybir.dt.float16, kind="ExternalInput"
    )
    xh = nc.dram_tensor(
        "xh", [NG2, P, NB2 * W2], mybir.dt.float16, kind="ExternalInput"
    )
    al = nc.dram_tensor(
        "al", [NG1, P, NB1 * W1], mybir.dt.bfloat16, kind="ExternalOutput"
    )
    ah = nc.dram_tensor(
        "ah", [NG2, P, NB2 * W2], mybir.dt.bfloat16, kind="ExternalOutput"
    )
    with TileContext(nc) as tc:
        with tc.tile_pool(name="sbuf", bufs=8) as pool:
            # heavy first (fills pipe); chunked mul, 1 Pool chunk
            _emit_tier(nc, pool, mybir, xh, ah, NG2, NB2, W2,
                       periodic_groups=set(range(NG2)), pool_chunks=(),
                       split_first=True)
            _emit_tier(nc, pool, mybir, xl, al, NG1, NB1, W1,
                       periodic_groups=set(range(NG1)), pool_chunks=(),
                       split_first=False)
    nc.compile()
    return nc


def _build_flat(w):
    import concourse.mybir as mybir
    from concourse import bacc
    from concourse.tile import TileContext

    nb, ng = 7, 14
    nc = bacc.Bacc(None, target_bir_lowering=False)
    x_in = nc.dram_tensor(
        "x", [ng, P, nb * w], mybir.dt.float16, kind="ExternalInput"
    )
    a_out = nc.dram_tensor(
        "alpha", [ng, P, nb * w], mybir.dt.bfloat16, kind="ExternalOutput"
    )
    kr = dict(KREP)
    kr.setdefault(w, w // 4 if (w // 4) * 4 == w else w)
    KREP.update(kr)
    with TileContext(nc) as tc:
        with tc.tile_pool(name="sbuf", bufs=8) as pool:
            _emit_tier(nc, pool, mybir, x_in, a_out, ng, nb, w,
                       periodic_groups=set(range(ng)), pool_chunks=(),
                       split_first=True)
    nc.compile()
    return nc


def kernel(e, edge_index, num_nodes):
    from concourse.bass_utils import run_bass_kernel_spmd
    import concourse.mybir as mybir

    e = np.ascontiguousarray(np.asarray(e, dtype=np.float32))
    tgt = np.asarray(edge_index)[1].astype(np.int32)
    E = e.shape[0]
    assert int(num_nodes) <= NPAD

    counts = np.bincount(tgt, minlength=NPAD).astype(np.int64)
    order = np.argsort(tgt, kind="stable")
    tgt_sorted = tgt[order]
    starts = np.zeros(NPAD + 1, dtype=np.int64)
    np.cumsum(counts, out=starts[1:])
    pos = np.arange(E, dtype=np.int64) - starts[tgt_sorted]
    e_sorted16 = e[order].astype(np.float16)
    bf16 = mybir.dt.np(mybir.dt.bfloat16)

    rank = np.argsort(-counts, kind="stable")
    two_tier = counts[rank[0]] <= W2 and counts[rank[K2]] <= W1

    if two_tier:
        if "2t" not in _CACHE:
            _CACHE["2t"] = _build_two_tier()
        nc = _CACHE["2t"]

        core_of = np.empty(NPAD, dtype=np.int64)
        row_of = np.empty(NPAD, dtype=np.int64)
        tier_of = np.zeros(NPAD, dtype=np.int8)
        hn, ln = rank[:K2], rank[K2:]
        core_of[hn] = np.arange(K2) % NCORES
        row_of[hn] = np.arange(K2) // NCORES
        tier_of[hn] = 1
        core_of[ln] = np.arange(K1) % NCORES
        row_of[ln] = np.arange(K1) // NCORES

        light = np.full((NCORES * T1 * P, W1), -60.0, dtype=np.float16)
        heavy = np.full((NCORES * T2 * P, W2), -60.0, dtype=np.float16)
        et = tier_of[tgt_sorted] == 1
        grow = core_of[tgt_sorted] * (T1 * P) + row_of[tgt_sorted]
        hrow = core_of[tgt_sorted] * (T2 * P) + row_of[tgt_sorted]
        light[grow[~et], pos[~et]] = e_sorted16[~et]
        heavy[hrow[et], pos[et]] = e_sorted16[et]

        # row ((g*nb + b)*P + p), col wcol -> DRAM [g, P, wcol, b]
        lt = light.reshape(NCORES, NG1, NB1, P, W1).transpose(0, 1, 3, 4, 2)
        ht = heavy.reshape(NCORES, NG2, NB2, P, W2).transpose(0, 1, 3, 4, 2)
        in_maps = [
            {
                "xl": np.ascontiguousarray(lt[c]).reshape(NG1, P, NB1 * W1),
                "xh": np.ascontiguousarray(ht[c]).reshape(NG2, P, NB2 * W2),
            }
            for c in range(NCORES)
        ]
        res = run_bass_kernel_spmd(nc, in_maps, core_ids=list(range(NCORES)))

        aln = np.empty((NCORES, NG1, P, W1, NB1), dtype=bf16)
        ahn = np.empty((NCORES, NG2, P, W2, NB2), dtype=bf16)
        for c in range(NCORES):
            aln[c] = np.asarray(res.results[c]["al"]).reshape(NG1, P, W1, NB1)
            ahn[c] = np.asarray(res.results[c]["ah"]).reshape(NG2, P, W2, NB2)
        lflat = aln.transpose(0, 1, 4, 2, 3).reshape(NCORES * T1 * P, W1)
        hflat = ahn.transpose(0, 1, 4, 2, 3).reshape(NCORES * T2 * P, W2)

        a_sorted = np.empty(E, dtype=np.float32)
        a_sorted[~et] = lflat[grow[~et], pos[~et]].astype(np.float32)
        a_sorted[et] = hflat[hrow[et], pos[et]].astype(np.float32)
        alpha = np.empty(E, dtype=np.float32)
        alpha[order] = a_sorted
        return alpha

    # fallback: flat interleaved layout
    max_deg = int(counts.max())
    w = max(352, -(-max_deg // 32) * 32)
    if ("flat", w) not in _CACHE:
        _CACHE[("flat", w)] = _build_flat(w)
    nc = _CACHE[("flat", w)]
    ng, nb = 14, 7

    padded = np.full((NPAD, w), -60.0, dtype=np.float16)
    padded[tgt_sorted, pos] = e_sorted16
    per_core = padded.reshape(NCORES, ng, nb, P, w).transpose(0, 1, 3, 4, 2)
    in_maps = [
        {"x": np.ascontiguousarray(per_core[c]).reshape(ng, P, nb * w)}
        for c in range(NCORES)
    ]
    res = run_bass_kernel_spmd(nc, in_maps, core_ids=list(range(NCORES)))
    out = np.empty((NCORES, ng, P, w, nb), dtype=bf16)
    for c in range(NCORES):
        out[c] = np.asarray(res.results[c]["alpha"]).reshape(ng, P, w, nb)
    alpha_padded = out.transpose(0, 1, 4, 2, 3).reshape(NPAD, w)
    alpha = np.empty(E, dtype=np.float32)
    alpha[order] = alpha_padded[tgt_sorted, pos].astype(np.float32)
    return alpha
